# revision 26
# baseline (speedup 1.0000x reference)
"""Causal single-head attention (4096x2048, d=128) on 8 TRN2 NeuronCores.

Strategy (flash-style sequence parallelism, v2):
- Q rows sharded mod-8 across cores (identical causal work profile per
  core); K/V projections sharded by contiguous 512-key blocks, projected
  K^T (fp8e4) and V (bf16->fp8?no: bf16) AllGathered.
- Xq/Xk inputs and Wq/Wk weights are quantized to fp8e4 on the host
  (weights pre-scaled by 64 to stay in normal fp8 range); the Q/K
  projections run in DoubleRow fp8 perf mode (2 fp8 MACs/cell/cycle,
  contraction 256 per matmul) - about 1.8x the bf16 projection rate and
  half the input DMA traffic.
- V projection is computed X-stationary: out[s,d] = sum_m X^T[m,s]^T W^T[m,d]
  so V lands directly in [keys, d] layout - no PE transposes - and the
  gather input is written contiguously.  V stays bf16 end-to-end (fp8 V
  costs ~2.6e-2 rel err, over the 2e-2 budget).
- The softmax-denominator ones-column is carried *inside the gathered V
  blocks* (each core writes [128, 4, 1+128] with col 0 = 1.0), so the
  gathered V streams straight into PV matmuls with FD=129 and no
  receiver-side fixups, and the gather reload runs 1032B-contiguous.
- Gathered K^T is consumed directly as the fp8 stationary operand of the
  score matmuls (mixed fp8 x bf16 matmul) - no on-chip upconvert.
- All inputs are host-swizzled into the exact SBUF layout ([128, t, n],
  m = 128 t + p) so every input DMA is fully contiguous per partition.
- Emission is software-pipelined 3 deep (loads(it+1) | proj(it) |
  attn(it-1)) and the PE stream interleaves score-group matmuls of
  attn(it-1) between projection chunks of proj(it), so the ScalarE exp
  chain (~11.5us/iter, the score-phase pacer) runs entirely under PE
  projection/PV work instead of stalling it.
"""

import math
import sys

sys.path.insert(0, "/opt/trn_rl_repo")

import ml_dtypes
import numpy as np

import concourse.bass as bass
import concourse.tile as tile
from concourse import bacc, mybir
from concourse.bass import ts
from concourse.bass_utils import run_bass_kernel_spmd

N_CORES = 8
SEQ = 4096
D_MODEL = 2048
D_HEAD = 128
R = SEQ // N_CORES          # 512 query rows per core
KB = SEQ // N_CORES         # 512 keys projected per core
N_QT = R // 128             # 4 query tiles of 128 rows per core
N_MT = D_MODEL // 128       # 16 contraction tiles for projections
N_KT = SEQ // 128           # 32 key tiles total
INV_SQRT = 1.0 / math.sqrt(D_MODEL)
WSCALE = 64.0               # fp8 weight pre-scale (host side)
VW = 129                    # gathered V block width: 1 ones-col + 128 dims

BF16 = mybir.dt.bfloat16
F32 = mybir.dt.float32
F8 = mybir.dt.float8e4
U8 = mybir.dt.uint8
DR = mybir.MatmulPerfMode.DoubleRow


def _build(reps=1, single=False, do_loads=True, do_proj=True, do_coll=True,
           do_reload=True, do_attn=True, do_coll_k=None, do_coll_v=None,
           cc_v_dtype="u8"):
    """single=True: 1-device build with collectives replaced by local
    copies — for TimelineSim occupancy analysis only.  The do_* flags are
    timing-ablation knobs (outputs are garbage when any is False)."""
    if do_coll_k is None:
        do_coll_k = do_coll
    if do_coll_v is None:
        do_coll_v = do_coll
    n_dev = 1 if single else N_CORES
    nc = bacc.Bacc("TRN2", target_bir_lowering=False, debug=False,
                   num_devices=n_dev)

    xq_d = nc.dram_tensor("xq8", [128, N_MT * R], F8, kind="ExternalInput").ap()
    xk_d = nc.dram_tensor("xk8", [128, N_MT * KB], F8, kind="ExternalInput").ap()
    xv_d = nc.dram_tensor("xv", [128, N_MT * KB], BF16, kind="ExternalInput").ap()
    wq_d = nc.dram_tensor("wq8", [128, N_MT * D_HEAD], F8, kind="ExternalInput").ap()
    wk_d = nc.dram_tensor("wk8", [128, N_MT * D_HEAD], F8, kind="ExternalInput").ap()
    wv_d = nc.dram_tensor("wv", [128, N_MT * D_HEAD], BF16, kind="ExternalInput").ap()
    qkb_d = nc.dram_tensor("qkbias", [D_HEAD, 2], F32, kind="ExternalInput").ap()
    vb_d = nc.dram_tensor("vbias", [128, 4 * D_HEAD], BF16, kind="ExternalInput").ap()
    mask_d = nc.dram_tensor("mask", [128, 8 * 128], BF16, kind="ExternalInput").ap()
    out_d = nc.dram_tensor("out", [R, D_HEAD], BF16, kind="ExternalOutput").ap()

    with tile.TileContext(nc) as tc:
        with (
            tc.tile_pool(name="const", bufs=1) as const,
            tc.tile_pool(name="xin", bufs=2) as xin,
            tc.tile_pool(name="work", bufs=2) as work,
            tc.tile_pool(name="psum", bufs=2, space="PSUM") as psum,
            tc.tile_pool(name="dram", bufs=2, space="DRAM") as dram,
        ):
            # ---- constants (loaded once, amortized across reps) ----
            wq8 = const.tile([128, N_MT, D_HEAD], F8, name="wq8")
            nc.sync.dma_start(wq8[:], wq_d.rearrange("p (t d) -> p t d", t=N_MT))
            wk8 = const.tile([128, N_MT, D_HEAD], F8, name="wk8")
            nc.sync.dma_start(wk8[:], wk_d.rearrange("p (t d) -> p t d", t=N_MT))
            wv_sb = const.tile([128, N_MT, D_HEAD], BF16, name="wv_sb")
            nc.sync.dma_start(wv_sb[:], wv_d.rearrange("p (t d) -> p t d", t=N_MT))
            qkb = const.tile([D_HEAD, 2], F32, name="qkb")
            nc.sync.dma_start(qkb[:], qkb_d[:])
            vbias = const.tile([128, 4 * D_HEAD], BF16, name="vbias")
            nc.sync.dma_start(vbias[:], vb_d[:])
            mask_sb = const.tile([128, 8 * 128], BF16, name="mask_sb")
            nc.sync.dma_start(mask_sb[:], mask_d[:])
            # Persistent double-buffered gathered-V tiles [s, kt, 1+128].
            # The ones column (softmax denominator) is baked in ONCE here;
            # per-iteration reloads overwrite only the V lanes.  Keeping the
            # ones out of the collective keeps its payload a power of two
            # (1032B/partition measured 26x slower than 1024B on the CCE).
            va_slots = []
            for i in range(2):
                vas = const.tile([128, N_KT, VW], BF16, name=f"va{i}")
                nc.vector.memset(vas.rearrange("p t x -> p (t x)"), 1.0)
                va_slots.append(vas)

            def emit_loads(it):
                xq = xin.tile([128, N_MT, R], F8, name="xq", tag="xq")
                xk = xin.tile([128, N_MT, KB], F8, name="xk", tag="xk")
                xv = xin.tile([128, N_MT, KB], BF16, name="xv", tag="xv")
                if do_loads:
                    nc.sync.dma_start(
                        xq[:], xq_d.rearrange("p (t r) -> p t r", t=N_MT))
                    nc.scalar.dma_start(
                        xk[:], xk_d.rearrange("p (t s) -> p t s", t=N_MT))
                    xv_src = xv_d.rearrange("p (t s) -> p t s", t=N_MT)
                    nc.sync.dma_start(xv[:, 0:8], xv_src[:, 0:8])
                    nc.scalar.dma_start(xv[:, 8:16], xv_src[:, 8:16])
                return (xq, xk, xv)

            def make_proj_chunks(it, loads):
                """8 emission chunks for iteration `it`'s projections +
                collectives; returns (chunks, handles-for-attn)."""
                xq, xk, xv = loads
                qt = work.tile([128, R], BF16, name="qt", tag="qt")
                kt8 = work.tile([128, KB], F8, name="kt8", tag="kt8")
                vt = work.tile([128, 4, D_HEAD], BF16, name="vt", tag="vt")
                ktf8 = work.tile([128, N_CORES, KB], F8, name="ktf8", tag="ktf8")
                va = va_slots[it % 2]

                cc_k_in = dram.tile([128, KB], F8, name="cc_k_in")
                cc_k_out = dram.tile([N_CORES, 128, KB], F8,
                                     addr_space="Shared", name="cc_k_out")
                # V gather ships as 1-byte-typed *bytes* (bitcast, not a
                # conversion) when cc_v_dtype != bf16.  NOTE: f8 is FAST but
                # CORRUPTS data (CCE canonicalizes fp8 NaN byte patterns) -
                # timing experiments only.
                cdt = {"u8": U8, "f8": F8, "bf16": BF16}[cc_v_dtype]
                cw = 4 * D_HEAD if cc_v_dtype == "bf16" else 8 * D_HEAD
                cc_v_in = dram.tile([128, cw], cdt, name="cc_v_in")
                cc_v_out = dram.tile([N_CORES, 128, cw], cdt,
                                     addr_space="Shared", name="cc_v_out")

                state = {}

                def q0():
                    pq = psum.tile([128, R], F32, name="pq", tag="proj")
                    state["pq"] = pq
                    for i in range(4 if do_proj else 0):
                        nc.tensor.matmul(
                            pq[:], wq8[:, 2 * i:2 * i + 2, :],
                            xq[:, 2 * i:2 * i + 2, :],
                            start=(i == 0), stop=False, perf_mode=DR)

                def q1():
                    pq = state["pq"]
                    for i in range(4, 8) if do_proj else []:
                        nc.tensor.matmul(
                            pq[:], wq8[:, 2 * i:2 * i + 2, :],
                            xq[:, 2 * i:2 * i + 2, :],
                            start=False, stop=(i == 7), perf_mode=DR)
                    if do_proj:
                        nc.vector.tensor_scalar(
                            qt[:], pq[:], 1.0 / WSCALE, qkb[:, 0:1],
                            op0=mybir.AluOpType.mult, op1=mybir.AluOpType.add)
                    elif do_attn:
                        nc.vector.memset(qt[:], 0.25)   # ablation stub

                def k0():
                    pk = psum.tile([128, KB], F32, name="pk", tag="proj")
                    state["pk"] = pk
                    for i in range(4 if do_proj else 0):
                        nc.tensor.matmul(
                            pk[:], wk8[:, 2 * i:2 * i + 2, :],
                            xk[:, 2 * i:2 * i + 2, :],
                            start=(i == 0), stop=False, perf_mode=DR)

                def k1():
                    pk = state["pk"]
                    for i in range(4, 8) if do_proj else []:
                        nc.tensor.matmul(
                            pk[:], wk8[:, 2 * i:2 * i + 2, :],
                            xk[:, 2 * i:2 * i + 2, :],
                            start=False, stop=(i == 7), perf_mode=DR)
                    if do_proj:
                        nc.vector.tensor_scalar(
                            kt8[:], pk[:], 1.0 / WSCALE, qkb[:, 1:2],
                            op0=mybir.AluOpType.mult, op1=mybir.AluOpType.add)
                    elif do_coll_k:
                        nc.vector.memset(kt8[:], 0.25)   # ablation stub
                    if do_coll_k:
                        nc.sync.dma_start(cc_k_in[:], kt8[:])
                        if single:
                            nc.sync.dma_start(cc_k_out[0], cc_k_in[:])
                        else:
                            nc.gpsimd.collective_compute(
                                "AllGather", mybir.AluOpType.bypass,
                                replica_groups=[list(range(N_CORES))],
                                ins=[cc_k_in.opt()], outs=[cc_k_out.opt()],
                            )

                def make_v(b):
                    def v():
                        if b == 0:
                            state["pv"] = psum.tile([128, 512], F32,
                                                    name="pv", tag="proj")
                        pv = state["pv"]
                        for t in range(N_MT if do_proj else 0):
                            nc.tensor.matmul(
                                pv[:, ts(b, 128)], xv[:, t, :][:, ts(b, 128)],
                                wv_sb[:, t, :],
                                start=(t == 0), stop=(t == N_MT - 1))
                        if b == 3:
                            if do_proj:
                                nc.vector.tensor_add(
                                    vt[:],
                                    pv.rearrange("p (t d) -> p t d", t=4),
                                    vbias.rearrange("p (t d) -> p t d", t=4))
                            elif do_coll_v:
                                nc.vector.memset(
                                    vt.rearrange("p t d -> p (t d)"),
                                    0.25)   # ablation stub
                            if do_coll_v:
                                src_ap = vt.rearrange("p t d -> p (t d)")
                                if cc_v_dtype != "bf16":
                                    src_ap = src_ap.bitcast(cdt)
                                nc.sync.dma_start(cc_v_in[:], src_ap)
                                if single:
                                    nc.sync.dma_start(cc_v_out[0], cc_v_in[:])
                                else:
                                    nc.gpsimd.collective_compute(
                                        "AllGather", mybir.AluOpType.bypass,
                                        replica_groups=[list(range(N_CORES))],
                                        ins=[cc_v_in.opt()],
                                        outs=[cc_v_out.opt()],
                                    )
                    return v

                # Reloads go through HWDGE (SWDGE descgen on gpsimd costs
                # ~3.7us each and saturates the Q7 sequencer).  reload_k is
                # emitted mid-tick on sync (its wait-on-CC resolves before
                # anything urgent queues behind it); reload_v is emitted at
                # the very END of the tick on scalar, after all exps, so its
                # CC wait blocks only next-tick prefetches.
                def reload_k():
                    if do_reload:
                        nc.sync.dma_start(
                            ktf8[:], cc_k_out.rearrange("r p s -> p r s"))
                    elif do_attn:
                        nc.vector.memset(ktf8.rearrange("p r s -> p (r s)"),
                                         0.25)   # ablation stub

                def reload_v():
                    if do_reload:
                        for b in range(4):
                            w = D_HEAD if cc_v_dtype == "bf16" else 2 * D_HEAD
                            s_ap = (cc_v_out[:, :, ts(b, w)]
                                    .rearrange("r p d -> p r d"))
                            if cc_v_dtype != "bf16":
                                s_ap = s_ap.bitcast(BF16)
                            nc.scalar.dma_start(va[:, b::4, 1:VW], s_ap)
                    elif do_attn:
                        nc.vector.memset(
                            va[:, :, 1:VW].rearrange("p t d -> p (t d)"),
                            0.25)   # ablation stub

                chunks = [q0, q1, k0, k1] + [make_v(b) for b in range(4)]
                return chunks, (reload_k, reload_v), (qt, ktf8, va)

            def make_attn_chunks(it, handles):
                """14 emission chunks for iteration `it`'s attention."""
                qt, ktf8, va = handles
                ktf = ktf8.rearrange("p r s -> p (r s)")
                p_all = work.tile([128, 10 * 1024], BF16, name="p_all",
                                  tag="p_all")
                slots = {}
                state = {"slot": 0}

                def make_sc(j, g):
                    def sc():
                        spsum = psum.tile([128, 1024], F32, name="spsum",
                                          tag="sc")
                        for q in range(8):
                            kt = 8 * g + q
                            nc.tensor.matmul(
                                spsum[:, ts(q, 128)],
                                ktf[:, ts(kt, 128)],
                                qt[:, ts(j, 128)],
                                start=True, stop=True)
                        slot = state["slot"]
                        state["slot"] += 1
                        slots[(j, g)] = slot
                        p_sb = p_all[:, ts(slot, 1024)]
                        nc.scalar.activation(p_sb, spsum[:],
                                             mybir.ActivationFunctionType.Exp,
                                             scale=INV_SQRT)
                        if g == j:
                            nc.vector.tensor_mul(p_sb, p_sb, mask_sb[:])
                    return sc

                def make_pv(j):
                    def pv():
                        opsum = psum.tile([128, VW], F32, name="opsum",
                                          tag="acc")
                        for g in range(j + 1):
                            p_sb = p_all[:, ts(slots[(j, g)], 1024)]
                            for q in range(8):
                                kt = 8 * g + q
                                nc.tensor.matmul(
                                    opsum[:],
                                    p_sb[:, ts(q, 128)].opt(),
                                    va[:, kt, :],
                                    start=(g == 0 and q == 0),
                                    stop=(g == j and q == 7))
                        recip = work.tile([128, 1], F32, name="recip",
                                          tag="recip")
                        nc.vector.reciprocal(recip[:], opsum[:, 0:1])
                        o_sb = work.tile([128, D_HEAD], BF16, name="o_sb",
                                         tag="o", bufs=4)
                        nc.vector.tensor_scalar_mul(o_sb[:], opsum[:, 1:VW],
                                                    recip[:])
                        nc.sync.dma_start(out_d[ts(j, 128), :], o_sb[:])
                    return pv

                sc = {(j, g): make_sc(j, g) for j in range(N_QT)
                      for g in range(j + 1)}
                pv = [make_pv(j) for j in range(N_QT)]
                return [sc[(0, 0)], sc[(1, 0)], sc[(1, 1)], sc[(2, 0)],
                        sc[(2, 1)], sc[(2, 2)], sc[(3, 0)], sc[(3, 1)],
                        pv[0], sc[(3, 2)], sc[(3, 3)], pv[1], pv[2], pv[3]]

            # ---- 3-deep software pipeline driver ----
            loads = emit_loads(0)
            attn_prev = None
            for it in range(reps):
                next_loads = emit_loads(it + 1) if it + 1 < reps else None
                pchunks, (rel_k, rel_v), handles = make_proj_chunks(it, loads)
                achunks = make_attn_chunks(it - 1, attn_prev) \
                    if (attn_prev is not None and do_attn) else []
                if achunks:
                    # interleave: sc groups between proj chunks, PV at tail
                    seq = []
                    for i in range(8):
                        seq.append(achunks[i])
                        seq.append(pchunks[i])
                    seq += [rel_k] + achunks[8:] + [rel_v]
                else:
                    seq = pchunks + [rel_k, rel_v]
                for c in seq:
                    c()
                attn_prev = handles
                loads = next_loads
            if do_attn:
                for c in make_attn_chunks(reps - 1, attn_prev):
                    c()

    nc.compile()
    return nc


_NC_CACHE = None


def _get_nc():
    global _NC_CACHE
    if _NC_CACHE is None:
        _NC_CACHE = _build()
    return _NC_CACHE


def _swizzle(a):
    """[2048, n] -> [128, 16*n] with row m = 128 t + p at [p, t*n : t*n+n]."""
    n = a.shape[1]
    return np.ascontiguousarray(
        a.reshape(N_MT, 128, n).transpose(1, 0, 2).reshape(128, N_MT * n))


def make_in_maps(input_q, input_k, input_v, WQ_w, WQ_b, WK_w, WK_b, WV_w, WV_b):
    bf16 = ml_dtypes.bfloat16
    f8 = ml_dtypes.float8_e4m3
    input_q = np.asarray(input_q, dtype=np.float32)
    input_k = np.asarray(input_k, dtype=np.float32)
    input_v = np.asarray(input_v, dtype=np.float32)
    wq8 = _swizzle(np.asarray(WQ_w, np.float32).T * WSCALE).astype(f8)
    wk8 = _swizzle(np.asarray(WK_w, np.float32).T * WSCALE).astype(f8)
    wv = _swizzle(np.asarray(WV_w, np.float32).T).astype(bf16)
    qkbias = np.stack([np.asarray(WQ_b, np.float32),
                       np.asarray(WK_b, np.float32)], axis=1)  # [128, 2]
    vbias = np.ascontiguousarray(np.broadcast_to(
        np.asarray(WV_b, np.float32)[None, None, :],
        (128, 4, D_HEAD)).reshape(128, 4 * D_HEAD)).astype(bf16)

    s = np.arange(128)[:, None, None]     # s_local (key within tile)
    m = np.arange(8)[None, :, None]       # diag-band key tile index
    r = np.arange(128)[None, None, :]     # r_local (query within tile)
    in_maps = []
    for c in range(N_CORES):
        mask_c = np.ascontiguousarray(
            ((128 * m + s) <= (8 * r + c))
        ).astype(bf16).reshape(128, 1024)
        in_maps.append({
            "xq8": _swizzle(input_q[c::8].T).astype(f8),
            "xk8": _swizzle(input_k[KB * c:KB * (c + 1)].T).astype(f8),
            "xv": _swizzle(input_v[KB * c:KB * (c + 1)].T).astype(bf16),
            "wq8": wq8, "wk8": wk8, "wv": wv,
            "qkbias": qkbias, "vbias": vbias,
            "mask": mask_c,
        })
    return in_maps


def assemble(results):
    full = np.empty((SEQ, D_HEAD), dtype=np.float32)
    for c in range(N_CORES):
        full[c::8] = results[c]["out"].astype(np.float32)
    return full


def kernel(**inputs):
    nc = _get_nc()
    in_maps = make_in_maps(**inputs)
    try:
        res = run_bass_kernel_spmd(nc, in_maps, core_ids=list(range(N_CORES)))
    except Exception:
        # The axon-tunneled devices occasionally report a transient
        # NRT_EXEC_UNIT_UNRECOVERABLE fault left over from a previous
        # session; a single retry has been observed to clear it.
        import time as _time
        _time.sleep(2.0)
        res = run_bass_kernel_spmd(nc, in_maps, core_ids=list(range(N_CORES)))
    return assemble(res.results)
